# revision 1
# baseline (speedup 1.0000x reference)
"""Trainium2 Bass kernel for nn_MetaOpPolicyNet_45749991637043 (histogram_binning).

kernel(**inputs) takes the FULL inputs (grid [4096,128,128] int32 + MLP weights),
shards the batch across 8 NeuronCores (pure data parallel, 512 batches/core),
and returns the FULL [4096, 32] float32 output.

Per-core design:
  - DMA-cast grid chunk (SWDGE int32->bf16) into SBUF [128(y), 128(batch), 128(x)]
  - DVE tensor_scalar is_equal per color -> bf16 0/1 mask (4x perf mode)
  - PE: for each x-column j, matmul with stationary [1 | y | j] ([128,3] bf16)
    accumulating over j in PSUM -> [3, batch] = (count, ysum, xsum) per batch,
    exactly (all integer arithmetic below 2^24 in fp32).
    Color 9 recovered by subtraction from constant per-batch totals.
  - means (max(cnt,1), reciprocal) + 40->64->32->32 MLP fully on-chip in fp32.
  - Output [32, 512] per core; host concatenates + transposes.
"""

import sys

for p in ("/opt/trn_rl_repo", "/root/.axon_site/_ro/trn_rl_repo"):
    if p not in sys.path:
        sys.path.insert(0, p)

import numpy as np
from contextlib import ExitStack

import concourse.bass as bass
import concourse.bacc as bacc
import concourse.tile as tile
from concourse import mybir
from concourse.bass_utils import run_bass_kernel_spmd

F32 = mybir.dt.float32
BF16 = mybir.dt.bfloat16
I32 = mybir.dt.int32
AF = mybir.ActivationFunctionType
ALU = mybir.AluOpType

H = 128
W = 128
NCOLORS = 10
N_CORES = 8


def _make_consts():
    import ml_dtypes

    y = np.arange(H, dtype=np.float32)
    j = np.arange(W, dtype=np.float32)
    wall = np.zeros((H, 3 * W), dtype=np.float32)
    wall[:, 0::3] = 1.0
    wall[:, 1::3] = y[:, None]
    wall[:, 2::3] = j[None, :]
    wall = wall.astype(ml_dtypes.bfloat16)

    sel = np.zeros((3, NCOLORS * 40), dtype=np.float32)
    for c in range(NCOLORS):
        base = 40 * c + 4 * c
        sel[0, base + 0] = 1.0
        sel[0, base + 1] = 1.0
        sel[1, base + 2] = 1.0
        sel[2, base + 3] = 1.0

    tot = np.array(
        [H * W, W * (H * (H - 1) // 2), H * (W * (W - 1) // 2)], dtype=np.float32
    ).reshape(3, 1)
    brd = np.array([[0.0, 1.0, 1.0]], dtype=np.float32)
    return {"wall": wall, "sel": sel, "tot": tot, "brd": brd}


def _build_nc(B, CB=128):
    assert B % CB == 0
    nchunks = B // CB

    nc = bacc.Bacc("TRN2", target_bir_lowering=False, debug=False)

    grid_d = nc.dram_tensor("grid", [B, H, W], I32, kind="ExternalInput")
    wall_d = nc.dram_tensor("wall", [H, 3 * W], BF16, kind="ExternalInput")
    sel_d = nc.dram_tensor("sel", [3, NCOLORS * 40], F32, kind="ExternalInput")
    tot_d = nc.dram_tensor("tot", [3, 1], F32, kind="ExternalInput")
    brd_d = nc.dram_tensor("brd", [1, 3], F32, kind="ExternalInput")
    w1_d = nc.dram_tensor("W1", [40, 64], F32, kind="ExternalInput")
    b1_d = nc.dram_tensor("b1", [64], F32, kind="ExternalInput")
    w2_d = nc.dram_tensor("W2", [64, 32], F32, kind="ExternalInput")
    b2_d = nc.dram_tensor("b2", [32], F32, kind="ExternalInput")
    w3_d = nc.dram_tensor("W3", [32, 32], F32, kind="ExternalInput")
    b3_d = nc.dram_tensor("b3", [32], F32, kind="ExternalInput")
    out_d = nc.dram_tensor("out", [32, B], F32, kind="ExternalOutput")

    with tile.TileContext(nc) as tc, ExitStack() as ctx:
        singles = ctx.enter_context(tc.tile_pool(name="singles", bufs=1))
        gpool = ctx.enter_context(tc.tile_pool(name="gpool", bufs=2))
        mpool = ctx.enter_context(tc.tile_pool(name="mpool", bufs=2))
        ppool = ctx.enter_context(
            tc.tile_pool(name="ppool", bufs=3, space=bass.MemorySpace.PSUM)
        )
        spool = ctx.enter_context(tc.tile_pool(name="spool", bufs=2))
        mlppsum = ctx.enter_context(
            tc.tile_pool(name="mlppsum", bufs=1, space=bass.MemorySpace.PSUM)
        )

        wall = singles.tile([H, 3 * W], BF16)
        nc.sync.dma_start(wall[:], wall_d[:])
        sel = singles.tile([3, NCOLORS * 40], F32)
        nc.sync.dma_start(sel[:], sel_d[:])
        tot = singles.tile([3, 1], F32)
        nc.sync.dma_start(tot[:], tot_d[:])
        brd = singles.tile([1, 3], F32)
        nc.sync.dma_start(brd[:], brd_d[:])
        w1 = singles.tile([40, 64], F32)
        nc.sync.dma_start(w1[:], w1_d[:])
        w2 = singles.tile([64, 32], F32)
        nc.sync.dma_start(w2[:], w2_d[:])
        w3 = singles.tile([32, 32], F32)
        nc.sync.dma_start(w3[:], w3_d[:])
        b1 = singles.tile([64, 1], F32)
        nc.sync.dma_start(b1[:], b1_d[:].rearrange("(n one) -> n one", one=1))
        b2 = singles.tile([32, 1], F32)
        nc.sync.dma_start(b2[:], b2_d[:].rearrange("(n one) -> n one", one=1))
        b3 = singles.tile([32, 1], F32)
        nc.sync.dma_start(b3[:], b3_d[:].rearrange("(n one) -> n one", one=1))

        for k in range(nchunks):
            b0 = k * CB
            gbf = gpool.tile([H, CB, W], BF16)
            # SWDGE dma with int32 -> bf16 cast; split to stay under the
            # 16384-descriptor-per-instruction limit
            nsub = max(1, (CB * H) // 4096)
            sb = CB // nsub
            for s in range(nsub):
                gsl = grid_d[b0 + s * sb : b0 + (s + 1) * sb, :, :].rearrange(
                    "b y x -> y b x"
                )
                nc.gpsimd.dma_start(out=gbf[:, s * sb : (s + 1) * sb, :], in_=gsl)

            # stats[s, c, b] : s in {cnt, ysum, xsum}
            stats = spool.tile([3, NCOLORS, CB], F32, tag="stats")
            for c in range(NCOLORS - 1):
                mask = mpool.tile([H, CB, W], BF16, tag="mask")
                nc.vector.tensor_scalar(
                    out=mask[:],
                    in0=gbf[:],
                    scalar1=float(c),
                    scalar2=None,
                    op0=ALU.is_equal,
                )
                ps = ppool.tile([3, CB], F32, tag="ps")
                for j in range(W):
                    nc.tensor.matmul(
                        ps[:],
                        wall[:, 3 * j : 3 * j + 3],
                        mask[:, :, j],
                        start=(j == 0),
                        stop=(j == W - 1),
                    )
                nc.scalar.copy(out=stats[:, c, :], in_=ps[:])

            # color 9 by subtraction: stats9 = tot - sum_{c<9}
            s9 = spool.tile([3, CB], F32, tag="s9")
            nc.vector.tensor_tensor(
                out=s9[:], in0=stats[:, 0, :], in1=stats[:, 1, :], op=ALU.add
            )
            for c in range(2, NCOLORS - 1):
                nc.vector.tensor_tensor(
                    out=s9[:], in0=s9[:], in1=stats[:, c, :], op=ALU.add
                )
            nc.vector.tensor_scalar(
                out=stats[:, NCOLORS - 1, :],
                in0=s9[:],
                scalar1=-1.0,
                scalar2=tot[:],
                op0=ALU.mult,
                op1=ALU.add,
            )

            # means: row broadcast [0,cnt,cnt] via K=1 matmuls (N<=512 fp32),
            # then max(.,1) per slice into denom
            denom = spool.tile([3, NCOLORS, CB], F32, tag="denom")
            cnt_flat = stats[0:1, :, :].rearrange("p c b -> p (c b)")
            den_flat = denom[:].rearrange("p c b -> p (c b)")
            tot_cb = NCOLORS * CB
            nslc = (tot_cb + 319) // 320
            slc = tot_cb // nslc
            assert slc * nslc == tot_cb and slc <= 512
            for i in range(nslc):
                cb_ps = mlppsum.tile([3, slc], F32, tag="cbps")
                nc.tensor.matmul(
                    cb_ps[:],
                    brd[:],
                    cnt_flat[:, i * slc : (i + 1) * slc],
                    start=True,
                    stop=True,
                )
                nc.vector.tensor_scalar(
                    out=den_flat[:, i * slc : (i + 1) * slc],
                    in0=cb_ps[:],
                    scalar1=1.0,
                    scalar2=None,
                    op0=ALU.max,
                )
            rec = spool.tile([3, NCOLORS, CB], F32, tag="rec")
            nc.vector.reciprocal(out=rec[:], in_=denom[:])
            statsm = spool.tile([3, NCOLORS, CB], F32, tag="statsm")
            nc.vector.tensor_tensor(
                out=statsm[:], in0=stats[:], in1=rec[:], op=ALU.mult
            )

            # X assembly via selector matmuls: X[40, CB]
            xp = mlppsum.tile([40, CB], F32, tag="xp")
            for c in range(NCOLORS):
                nc.tensor.matmul(
                    xp[:],
                    sel[:, 40 * c : 40 * (c + 1)],
                    statsm[:, c, :],
                    start=(c == 0),
                    stop=(c == NCOLORS - 1),
                )
            xsb = spool.tile([40, CB], F32, tag="xsb")
            nc.scalar.copy(out=xsb[:], in_=xp[:])

            # MLP
            h1p = mlppsum.tile([64, CB], F32, tag="h1")
            nc.tensor.matmul(h1p[:], w1[:], xsb[:], start=True, stop=True)
            h1s = spool.tile([64, CB], F32, tag="h1s")
            nc.scalar.activation(h1s[:], h1p[:], AF.Relu, bias=b1[:])

            h2p = mlppsum.tile([32, CB], F32, tag="h2")
            nc.tensor.matmul(h2p[:], w2[:], h1s[:], start=True, stop=True)
            h2s = spool.tile([32, CB], F32, tag="h2s")
            nc.scalar.activation(h2s[:], h2p[:], AF.Relu, bias=b2[:])

            h3p = mlppsum.tile([32, CB], F32, tag="h3")
            nc.tensor.matmul(h3p[:], w3[:], h2s[:], start=True, stop=True)
            osb = spool.tile([32, CB], F32, tag="osb")
            nc.scalar.activation(osb[:], h3p[:], AF.Identity, bias=b3[:])

            nc.sync.dma_start(out_d[:, b0 : b0 + CB], osb[:])

    nc.compile()
    return nc


_NC_CACHE = {}


def _get_nc(B):
    if B not in _NC_CACHE:
        _NC_CACHE[B] = _build_nc(B)
    return _NC_CACHE[B]


def kernel(grid, W1, b1, W2, b2, W3, b3, _trace=False, _trace_kwargs=None):
    grid = np.ascontiguousarray(np.asarray(grid, dtype=np.int32))
    B_total = grid.shape[0]
    assert B_total % N_CORES == 0
    Bc = B_total // N_CORES

    consts = _make_consts()
    common = {
        "wall": consts["wall"],
        "sel": consts["sel"],
        "tot": consts["tot"],
        "brd": consts["brd"],
        "W1": np.asarray(W1, dtype=np.float32),
        "b1": np.asarray(b1, dtype=np.float32),
        "W2": np.asarray(W2, dtype=np.float32),
        "b2": np.asarray(b2, dtype=np.float32),
        "W3": np.asarray(W3, dtype=np.float32),
        "b3": np.asarray(b3, dtype=np.float32),
    }
    in_maps = [
        {"grid": grid[i * Bc : (i + 1) * Bc], **common} for i in range(N_CORES)
    ]

    nc = _get_nc(Bc)
    kw = {}
    if _trace:
        kw = {"trace": True, "trace_kwargs": _trace_kwargs or {}}
    res = run_bass_kernel_spmd(nc, in_maps, core_ids=list(range(N_CORES)), **kw)
    outs = [np.asarray(r["out"], dtype=np.float32) for r in res.results]  # [32, Bc]
    full = np.concatenate(outs, axis=1).T  # [B_total, 32]
    out = np.ascontiguousarray(full, dtype=np.float32)
    if _trace:
        return out, res
    return out
